# revision 68
# baseline (speedup 1.0000x reference)
"""CARAFE (content-aware upsample, power-normalized softmax) on 8 TRN2 cores.

Math (reference.py): X (2,256,64,64) ->
  conv1x1(256->64) + bn + relu -> conv3x3(64->100) + bn -> pixel_shuffle(2)
  -> W (2,25,128,128) -> softmax(clip(W)^p) over 25 taps
  out[b,c,y,x] = sum_{ki,kj} W[b,(ki,kj),y,x] * Xpad[b,c,y//2+ki-2,x//2+kj-2]

Strategy (pure data-parallel over h, 8 low-res rows / core):
  * conv1x1 / conv3x3 as bf16 GEMMs (channels on partitions).
  * softmax via ACT transcendentals; tap-sums via a 100x4 selection matmul;
    reciprocal on [4,512]; denominator broadcast back to 100 partitions via
    a 4x100 selection matmul (no DRAM bounce).
  * The per-pixel 25-tap weighted sum is a banded matmul per output row h:
    out[c,(ry,x)] = sum_p XT_r[p,c] * B_ki[p,(ry,x)] accumulated over ki,
    where B_ki[w+kj, ry*128+2w+rx] = Wnorm[(ki,kj,ry,rx), h, w].  B is built
    ON-CHIP: PE-transpose Wnorm rows -> 5 partition-shift matmuls (constant
    shift matrices) -> one GPSIMD local_scatter per (b,h) placing the
    diagonal bands (per-partition indices, zeros implicit).
  * XT_r strips come from PE transposes of the input rows.

kernel(**inputs) takes the FULL inputs and returns the FULL output.
"""

import numpy as np
import ml_dtypes

SCALE = 2
K_UP = 5
B, C, H, W = 2, 256, 64, 64
N_CORES = 8
HS = H // N_CORES            # 8 low-res rows per core
XROWS = HS + 4               # 12 rows (with +-2 halo)
WP = W + 4                   # 68 (w padded by 2 each side)
CMID, CENC = 64, 100
NSLOT = K_UP * 2 * W * SCALE // 2  # bts columns per ki = 256
BTN = K_UP * 256             # 1280 elems per bts row
LCH = 80                     # local_scatter channels (68 rounded up to 16x)

_STATE = {}


def _build_nc():
    import concourse.bass as bass
    import concourse.tile as tile
    from concourse import mybir
    from concourse.vector_clock import ScopedClock
    from concourse.tile_rust import add_dep_helper

    # --- workaround: this walrus build rejects >1 sync-wait on CTRL-class
    # instructions; split the Tile tail-drain waits into 1-wait NOPs. ---
    def patched_drain_and_barrier(self, tick_clock, wait_clock):
        maxw = 1
        carrier = self.nc.sync.nop()
        wait_clock.add_sem_waits(
            carrier.ins, ScopedClock({None: tick_clock.global_clock})
        )
        si = carrier.ins.sync_info
        waits = list(si.on_wait) if si is not None else []
        if len(waits) > maxw:
            si.on_wait = waits[:maxw]
            carrier.ins.sync_info = si
            rest = waits[maxw:]
            for i in range(0, len(rest), maxw):
                n = self.nc.sync.nop()
                n.ins.sync_info = mybir.SyncInfo(
                    on_wait=rest[i : i + maxw], on_update=[]
                )
        self.nc.sync.drain()
        self.nc.all_engine_barrier()
        assert self.sems is not None
        popped = self.nc._tile_sem_poison_stack.pop()
        assert popped is self._sem_poison
        self.nc.clear_and_free_semaphores(list(self.sems.allocated().values()))
        self.nc.all_engine_barrier()

    tile.TileContext._drain_and_barrier = patched_drain_and_barrier

    # --- workaround #2: the same walrus build accepts at most ONE sync wait
    # on ANY instruction.  Post-process the serialized BIR: hoist excess
    # waits onto single-wait NoOps inserted just before, on the same engine
    # (same program point, so semantics are unchanged). ---
    import orjson

    def _split_waits_json(raw: bytes) -> bytes:
        j = orjson.loads(raw)
        n = 0
        changed = False
        for fn in j["functions"]:
            for bb in fn["blocks"]:
                out = []
                for ins in bb["instructions"]:
                    si = ins.get("sync_info")
                    waits = si.get("on_wait") if si else None
                    if waits and len(waits) > 1:
                        changed = True
                        for wt in waits[:-1]:
                            n += 1
                            out.append(
                                {
                                    "debug": ins.get("debug", 0),
                                    "engine": ins["engine"],
                                    "ins": [],
                                    "outs": [],
                                    "name": f"WSPL-{n}",
                                    "opcode": "NoOp",
                                    "sync_info": {"on_update": [], "on_wait": [wt]},
                                }
                            )
                        si["on_wait"] = [waits[-1]]
                    out.append(ins)
                bb["instructions"] = out
        return orjson.dumps(j) if changed else raw

    if not getattr(bass.Bass.to_json_bytes, "_wait_split", False):
        _orig_tjb = bass.Bass.to_json_bytes

        def patched_to_json_bytes(self):
            return _split_waits_json(_orig_tjb(self))

        patched_to_json_bytes._wait_split = True
        bass.Bass.to_json_bytes = patched_to_json_bytes

    f32 = mybir.dt.float32
    bf16 = mybir.dt.bfloat16
    i16 = mybir.dt.int16
    AF = mybir.ActivationFunctionType

    nc = bass.Bass()

    # ---- parameters ----
    # Packed inputs (host-prepared layouts; see _make_in_maps):
    #  xh    [B, 2, 128, XROWS*WP]   bf16  image, channels on partitions
    #  xtin  [B, XROWS, WP, 256]     bf16  image pre-transposed (w on partitions)
    #  bfpk  [128, BFPK]             bf16  ident|mask|shmat|encT|compT
    #  f32pk [CENC, F32PK]           f32   sel|selT|bn params
    #  lsidx [LCH, CENC]             i16   local_scatter indices
    BFPK = 128 + 660 + K_UP * LCH + 9 * CENC + 2 * CMID
    F32PK = 4 + CENC + 3
    xtin = nc.declare_dram_parameter(
        "xtin", [B, XROWS, WP, 256], bf16, isOutput=False
    )
    xc1 = nc.declare_dram_parameter(
        "xc1", [B, 128, 2, 2, 320], bf16, isOutput=False
    )
    bfpk = nc.declare_dram_parameter("bfpk", [128, BFPK], bf16, isOutput=False)
    f32pk = nc.declare_dram_parameter("f32pk", [CENC, F32PK], f32, isOutput=False)
    lsidx = nc.declare_dram_parameter("lsidx", [LCH, CENC], i16, isOutput=False)

    out = nc.declare_dram_parameter(
        "out", [B, C, 2 * HS, 2 * W], f32, isOutput=True
    )

    def dram_ap(param, offset, dims):
        return bass.AP(tensor=param, offset=offset, ap=[list(d) for d in dims])

    with tile.TileContext(nc) as tc:
        import contextlib

        ctx = contextlib.ExitStack()
        const = ctx.enter_context(tc.tile_pool(name="const", bufs=1))
        sm = ctx.enter_context(tc.tile_pool(name="sm", bufs=2))
        dp = ctx.enter_context(tc.tile_pool(name="dp", bufs=4))
        btp = ctx.enter_context(tc.tile_pool(name="btp", bufs=16))
        op = ctx.enter_context(tc.tile_pool(name="op", bufs=6))
        ps_big = ctx.enter_context(tc.tile_pool(name="ps_big", bufs=2, space="PSUM"))
        ps_bf = ctx.enter_context(tc.tile_pool(name="ps_bf", bufs=1, space="PSUM"))
        ps_sh = ctx.enter_context(tc.tile_pool(name="ps_sh", bufs=2, space="PSUM"))
        ps_e = ctx.enter_context(tc.tile_pool(name="ps_e", bufs=3, space="PSUM"))

        # ---- packed constants in SBUF (sync queue; scalar queue kept free
        # for ACT compute + out-DMAs).  conv1(b0) critical path first: weights
        # (tail of bfpk) as a separate early DMA, then f32pk, then the rest.
        bf_sb = const.tile([128, BFPK], bf16, tag="bfpk")
        W_OFF = 128 + 660 + K_UP * LCH  # encT|compT live at the tail
        nc.sync.dma_start(
            out=bf_sb[:, W_OFF:BFPK],
            in_=dram_ap(
                bfpk, W_OFF, [[BFPK, 128], [1, BFPK - W_OFF]]
            ),
        )
        f32_sb = const.tile([CENC, F32PK], f32, tag="f32pk")
        nc.sync.dma_start(out=f32_sb[:, :], in_=f32pk[:, :])

        o_id = 0
        ident_sb = bf_sb[:, 0:128]
        o_id += 128
        mask_v = bf_sb[0:CMID, o_id : o_id + 660].rearrange(
            "p (a b) -> p a b", b=66
        )
        o_id += 660
        sh_all = bf_sb[0:W, o_id : o_id + K_UP * LCH].rearrange(
            "p (a b) -> p a b", b=LCH
        )
        o_id += K_UP * LCH
        enc_bf = []
        for j in range(9):
            enc_bf.append(bf_sb[0:CMID, o_id : o_id + CENC])
            o_id += CENC
        comp_bf = []
        for ct in range(2):
            comp_bf.append(bf_sb[:, o_id : o_id + CMID])
            o_id += CMID

        sel_sb = f32_sb[:, 0:4]
        selT_sb = f32_sb[0:4, 4 : 4 + CENC]
        # bn folded into weights host-side; shifts + clipped power as columns
        shift1_ap = f32_sb[0:CMID, 4 + CENC : 4 + CENC + 1]
        shift2_ap = f32_sb[:, 4 + CENC + 1 : 4 + CENC + 2]
        pb_sb = f32_sb[:, 4 + CENC + 2 : 4 + CENC + 3]

        # ---- Y1 tiles (zeroed once; borders stay zero) ----
        y1 = []
        for b in range(B):
            t = const.tile([CMID, 10, 66], bf16, tag=f"y1_{b}")
            nc.vector.memset(t[:, :, :], 0.0)
            y1.append(t)

        # ---- per-batch X loads (sync queue, emitted lazily per b) ----
        xts_all = [None, None]
        xc1_sb = [None, None]

        def xload(b, rest=False):
            if not rest:
                t = const.tile([128, 2, 2, 320], bf16, tag=f"xc1{b}")
                nc.sync.dma_start(
                    out=t[:, :, :, :],
                    in_=dram_ap(xc1, b * 128 * 1280, [[1280, 128], [1, 1280]]),
                )
                xc1_sb[b] = t
                return
            t = const.tile([WP, XROWS, 256], bf16, tag=f"xts{b}")
            nc.sync.dma_start(
                out=t[:, :, :],
                in_=dram_ap(
                    xtin,
                    b * XROWS * WP * 256,
                    [[256, WP], [WP * 256, XROWS], [1, 256]],
                ),
            )
            xts_all[b] = t

        en_sbs = {}
        bts_alls = {}

        def prep(b):
            # ===== conv1x1 + bn1 + relu (contiguous host-packed input) =====
            for half in range(2):
                pcb = ps_big.tile([CENC, HS * W], f32, tag="big")
                pc = pcb[0:CMID, 0:320]
                for ct in range(2):
                    nc.tensor.matmul(
                        pc,
                        comp_bf[ct],
                        xc1_sb[b][:, half, ct, :],
                        start=(ct == 0),
                        stop=(ct == 1),
                    )
                nc.vector.tensor_scalar(
                    y1[b][:, 5 * half : 5 * half + 5, 1 : 1 + W],
                    pc,
                    shift1_ap,
                    0.0,
                    mybir.AluOpType.add,
                    mybir.AluOpType.max,
                )
            # zero out-of-image rows / padding cols
            nc.vector.tensor_mul(y1[b][:, :, :], y1[b][:, :, :], mask_v)

            # ===== conv3x3 + bn2 =====
            pc3 = ps_big.tile([CENC, HS * W], f32, tag="big")
            jj = 0
            for dy in (-1, 0, 1):
                for dx in (-1, 0, 1):
                    nc.tensor.matmul(
                        pc3[:, :],
                        enc_bf[jj],
                        y1[b][:, 1 + dy : 9 + dy, 1 + dx : 1 + dx + W],
                        start=(jj == 0),
                        stop=(jj == 8),
                    )
                    jj += 1
            # ===== bn2-shift + clip (fused DVE) + power + softmax numerator ==
            w_sb = sm.tile([CENC, HS * W], f32, tag="w")
            nc.vector.tensor_scalar(
                w_sb[:, :],
                pc3[:, :],
                shift2_ap,
                1e-5,
                mybir.AluOpType.add,
                mybir.AluOpType.max,
            )
            nc.scalar.activation(w_sb[:, :], w_sb[:, :], AF.Ln)
            nc.scalar.activation(w_sb[:, :], w_sb[:, :], AF.Exp, scale=pb_sb)
            e_sb = sm.tile([CENC, HS * W], f32, tag="e")
            nc.scalar.activation(e_sb[:, :], w_sb[:, :], AF.Exp)

            # ===== tap-sums, reciprocal, broadcast, normalize =====
            psb = ps_big.tile([CENC, HS * W], f32, tag="big")
            ps = psb[0:4, :]
            nc.tensor.matmul(ps, sel_sb, e_sb[:, :], start=True, stop=True)
            r4_sb = sm.tile([4, HS * W], f32, tag="r4")
            nc.vector.reciprocal_approx_fast(r4_sb[:, :], ps)
            rb_ps = ps_big.tile([CENC, HS * W], f32, tag="big")
            nc.tensor.matmul(
                rb_ps[:, :], selT_sb, r4_sb[:, :], start=True, stop=True
            )
            en_sb = const.tile([CENC, HS, W], bf16, tag=f"en{b}")
            nc.vector.tensor_mul(
                en_sb[:, :, :],
                e_sb[:, :].rearrange("p (a b) -> p a b", b=W),
                rb_ps[:, :].rearrange("p (a b) -> p a b", b=W),
            )
            en_sbs[b] = en_sb

        def band(b):
            # ===== banded-matrix build (h-quads): 4 transposes into one psum,
            # 5 quad-fused shift matmuls, then per-h data copy + local_scatter
            # into per-pair [LCH, 2*BTN] tiles =====
            en_sb = en_sbs[b]
            bts_all = []
            for hq in range(HS // 4):
                tpt = ps_bf.tile([W, 4 * CENC], bf16, tag="bf")
                for hh in range(4):
                    nc.tensor.transpose(
                        tpt[:, CENC * hh : CENC * (hh + 1)],
                        en_sb[:, 4 * hq + hh, :],
                        ident_sb[0:CENC, 0:CENC],
                    )
                tp_q = dp.tile([W, 4 * CENC], bf16, tag="tps")
                nc.vector.tensor_copy(tp_q[:, :], tpt[:, :])
                # cols of tp_q: hh*100 + ki*20 + kj*4 + u
                tp_v = tp_q[:, :].rearrange(
                    "p (hh ki r) -> p hh ki r", hh=4, r=20
                )
                sh_ps = ps_sh.tile([LCH, 4 * CENC], f32, tag="sh")
                sh_v = sh_ps[:, :].rearrange("p (hh r) -> p hh r", hh=4)
                for s in range(K_UP):
                    # out cols (hh, 20) at base s*20; rhs (hh, ki, u) base s*4
                    nc.tensor.matmul(
                        sh_v[:, :, 20 * s : 20 * (s + 1)],
                        sh_all[:, s, :],
                        tp_v[:, :, :, 4 * s : 4 * (s + 1)],
                        start=True,
                        stop=True,
                    )
                for hp2 in range(2):
                    btsp = btp.tile([LCH, 2 * BTN], bf16, tag="bts")
                    for hh2 in range(2):
                        hh = 2 * hp2 + hh2
                        data_sb = dp.tile([LCH, CENC], bf16, tag="data")
                        nc.vector.tensor_copy(
                            data_sb[:, :], sh_ps[:, CENC * hh : CENC * (hh + 1)]
                        )
                        nc.gpsimd.local_scatter(
                            out_ap=btsp[:, BTN * hh2 : BTN * (hh2 + 1)],
                            data_ap=data_sb[:, :],
                            idxs_ap=lsidx_sb[:, :],
                            channels=LCH,
                            num_elems=BTN,
                            num_idxs=CENC,
                        )
                    bts_all.append(btsp)
            bts_alls[b] = bts_all

        def eins(b):
            # ===== banded einsum: h-pair-fused matmuls (6 per pair per ct),
            # one out DMA per (pair, ct) =====
            bts_all = bts_alls[b]
            for hp in range(HS // 2):
                ha = 2 * hp
                btsp = bts_all[hp]
                for ct in range(2):
                    pe = ps_e.tile([128, 512], f32, tag="pe")
                    cs = ct * 128
                    # fused strips jj=1..4: regions (ha@ki=jj, hb@ki=jj-1)
                    base_ap = btsp[0:WP, 0:256]
                    for jj in range(1, K_UP):
                        rhs = bass.AP(
                            tensor=base_ap.tensor,
                            offset=base_ap.offset + jj * 256,
                            ap=[list(base_ap.ap[0]), [BTN - 256, 2], [1, 256]],
                        )
                        nc.tensor.matmul(
                            pe[:, :],
                            xts_all[b][:, ha + jj, cs : cs + 128],
                            rhs,
                            start=(jj == 1),
                            stop=False,
                        )
                    # single strip jj=0 -> region A only
                    nc.tensor.matmul(
                        pe[:, 0:256],
                        xts_all[b][:, ha, cs : cs + 128],
                        btsp[0:WP, 0:256],
                        start=False,
                        stop=True,
                    )
                    # single strip jj=5 -> region B only
                    nc.tensor.matmul(
                        pe[:, 256:512],
                        xts_all[b][:, ha + 5, cs : cs + 128],
                        btsp[0:WP, BTN + 4 * 256 : BTN + 5 * 256],
                        start=False,
                        stop=True,
                    )
                    o_sb = op.tile([128, 512], f32, tag="osb")
                    nc.vector.tensor_copy(o_sb[:, :], pe[:, :])
                    oeng = nc.sync if ct == 0 else nc.scalar
                    oeng.dma_start(
                        out=dram_ap(
                            out,
                            b * C * 2 * HS * 2 * W
                            + ct * 128 * 2 * HS * 2 * W
                            + 4 * hp * 2 * W,
                            [[2 * HS * 2 * W, 128], [1, 512]],
                        ),
                        in_=o_sb[:, :],
                    )

        xload(0)
        # remaining constants + idx after the conv1(b0)-critical loads
        nc.sync.dma_start(
            out=bf_sb[:, 0:W_OFF],
            in_=dram_ap(bfpk, 0, [[BFPK, 128], [1, W_OFF]]),
        )
        lsidx_sb = const.tile([LCH, CENC], i16, tag="lsidx")
        nc.sync.dma_start(out=lsidx_sb[:, :], in_=lsidx[:, :])
        xload(1)
        xload(0, rest=True)
        xload(1, rest=True)
        prep(0)
        prep(1)
        band(0)
        band(1)
        eins(0)
        eins(1)

        ctx.close()

    # ---- Bacc-style finishing passes: library loads + ISA assembly ----
    from concourse.library_config import all_libraries, standard
    import bass_rust as _bass_rust

    lib_mask = {}
    for lib in all_libraries:
        for it in lib.instructions:
            lib_mask[it] = lib_mask.get(it, 0) | (1 << lib.index)
    _bass_rust.insert_library_loads(nc, lib_mask, len(all_libraries), standard.index)
    mybir.codegen_inst_isa_subclasses(nc)

    return nc


def _get_nc():
    if "nc" not in _STATE:
        _STATE["nc"] = _build_nc()
    return _STATE["nc"]


def _make_in_maps(inputs):
    bf16 = ml_dtypes.bfloat16
    BFPK = 128 + 660 + K_UP * LCH + 9 * CENC + 2 * CMID
    F32PK = 4 + CENC + 3
    X = np.asarray(inputs["X"], dtype=np.float32)
    Xp = np.pad(X, ((0, 0), (0, 0), (2, 2), (2, 2)))

    sel = np.zeros((CENC, 4), np.float32)
    for p in range(CENC):
        sel[p, p % 4] = 1.0
    shmat = np.zeros((K_UP, W, LCH), np.float32)
    for s in range(K_UP):
        for w in range(W):
            shmat[s, w, w + s] = 1.0
    lsidx = np.full((LCH, CENC), -1, np.int16)
    for p in range(WP):
        for s in range(K_UP):
            w = p - s
            if 0 <= w < W:
                for ki in range(K_UP):
                    for u in range(4):
                        ry, rx = u // 2, u % 2
                        c = s * 20 + ki * 4 + u
                        lsidx[p, c] = ki * 256 + ry * 128 + 2 * w + rx
    # fold eval-mode batchnorms into the conv weights
    inv1 = np.asarray(inputs["comp_gamma"], np.float32) / np.sqrt(
        np.asarray(inputs["comp_var"], np.float32) + 1e-5
    )
    shift1 = (
        np.asarray(inputs["comp_beta"], np.float32)
        - np.asarray(inputs["comp_mean"], np.float32) * inv1
    )
    inv2 = np.asarray(inputs["enc_gamma"], np.float32) / np.sqrt(
        np.asarray(inputs["enc_var"], np.float32) + 1e-5
    )
    shift2 = (
        np.asarray(inputs["enc_beta"], np.float32)
        - np.asarray(inputs["enc_mean"], np.float32) * inv2
    )
    comp_wT = (
        np.asarray(inputs["comp_w"], np.float32)[:, :, 0, 0] * inv1[:, None]
    ).T.reshape(2, 128, CMID)
    enc_wT = (
        np.asarray(inputs["enc_w"], np.float32) * inv2[:, None, None, None]
    ).reshape(CENC, CMID, 9).transpose(2, 1, 0)

    # bf16 pack: ident | y1mask(per-core) | shmat | encT | compT
    bfpk = np.zeros((128, BFPK), np.float32)
    o = 0
    bfpk[:, o : o + 128] = np.eye(128)
    o_mask = o = o + 128
    o += 660
    bfpk[0:W, o : o + K_UP * LCH] = shmat.transpose(1, 0, 2).reshape(W, K_UP * LCH)
    o += K_UP * LCH
    bfpk[0:CMID, o : o + 9 * CENC] = enc_wT.transpose(1, 0, 2).reshape(
        CMID, 9 * CENC
    )
    o += 9 * CENC
    bfpk[:, o : o + 2 * CMID] = comp_wT.transpose(1, 0, 2).reshape(128, 2 * CMID)

    # f32 pack: sel | selT | shift1 | shift2 | clipped power (broadcast)
    f32pk = np.zeros((CENC, F32PK), np.float32)
    f32pk[:, 0:4] = sel
    f32pk[0:4, 4 : 4 + CENC] = sel.T
    f32pk[0:CMID, 4 + CENC] = shift1
    f32pk[:, 4 + CENC + 1] = shift2
    f32pk[:, 4 + CENC + 2] = max(
        float(np.asarray(inputs["power_p"], np.float32)[0]), 1e-5
    )

    common = {
        "f32pk": f32pk,
        "lsidx": lsidx,
    }
    in_maps = []
    for core in range(N_CORES):
        r0 = HS * core
        xh4 = np.ascontiguousarray(Xp[:, :, r0 : r0 + XROWS, :]).astype(bf16)
        mask = np.zeros((10, 66), np.float32)
        for rr in range(10):
            grow = r0 - 1 + rr
            if 0 <= grow < H:
                mask[rr, 1 : 1 + W] = 1.0
        bfpk_c = bfpk.copy()
        bfpk_c[0:CMID, o_mask : o_mask + 660] = mask.reshape(1, 660)
        xc1 = (
            xh4[:, :, 1:11, 2:66]
            .reshape(B, 2, 128, 2, 5, 64)
            .transpose(0, 2, 3, 1, 4, 5)
            .reshape(B, 128, 2, 2, 320)
        )
        m = dict(common)
        m["xc1"] = np.ascontiguousarray(xc1)
        m["xtin"] = np.ascontiguousarray(xh4.transpose(0, 2, 3, 1))
        m["bfpk"] = bfpk_c.astype(bf16)
        in_maps.append(m)
    return in_maps


def _run(inputs, trace=False):
    from concourse.bass_utils import run_bass_kernel_spmd

    if trace:
        import sys, os
        sys.path.insert(0, os.path.dirname(os.path.abspath(__file__)))
        import hookshim  # noqa: F401

    nc = _get_nc()
    in_maps = _make_in_maps(inputs)
    res = run_bass_kernel_spmd(
        nc, in_maps, core_ids=list(range(N_CORES)), trace=trace
    )
    out = np.concatenate([res.results[c]["out"] for c in range(N_CORES)], axis=2)
    return out, res


def kernel(**inputs):
    out, _ = _run(inputs, trace=False)
    return out


# revision 69
# speedup vs baseline: 1.0081x; 1.0081x over previous
"""CARAFE (content-aware upsample, power-normalized softmax) on 8 TRN2 cores.

Math (reference.py): X (2,256,64,64) ->
  conv1x1(256->64) + bn + relu -> conv3x3(64->100) + bn -> pixel_shuffle(2)
  -> W (2,25,128,128) -> softmax(clip(W)^p) over 25 taps
  out[b,c,y,x] = sum_{ki,kj} W[b,(ki,kj),y,x] * Xpad[b,c,y//2+ki-2,x//2+kj-2]

Strategy (pure data-parallel over h, 8 low-res rows / core):
  * conv1x1 / conv3x3 as bf16 GEMMs (channels on partitions).
  * softmax via ACT transcendentals; tap-sums via a 100x4 selection matmul;
    reciprocal on [4,512]; denominator broadcast back to 100 partitions via
    a 4x100 selection matmul (no DRAM bounce).
  * The per-pixel 25-tap weighted sum is a banded matmul per output row h:
    out[c,(ry,x)] = sum_p XT_r[p,c] * B_ki[p,(ry,x)] accumulated over ki,
    where B_ki[w+kj, ry*128+2w+rx] = Wnorm[(ki,kj,ry,rx), h, w].  B is built
    ON-CHIP: PE-transpose Wnorm rows -> 5 partition-shift matmuls (constant
    shift matrices) -> one GPSIMD local_scatter per (b,h) placing the
    diagonal bands (per-partition indices, zeros implicit).
  * XT_r strips come from PE transposes of the input rows.

kernel(**inputs) takes the FULL inputs and returns the FULL output.
"""

import numpy as np
import ml_dtypes

SCALE = 2
K_UP = 5
B, C, H, W = 2, 256, 64, 64
N_CORES = 8
HS = H // N_CORES            # 8 low-res rows per core
XROWS = HS + 4               # 12 rows (with +-2 halo)
WP = W + 4                   # 68 (w padded by 2 each side)
CMID, CENC = 64, 100
NSLOT = K_UP * 2 * W * SCALE // 2  # bts columns per ki = 256
BTN = K_UP * 256             # 1280 elems per bts row
LCH = 80                     # local_scatter channels (68 rounded up to 16x)

_STATE = {}


def _build_nc():
    import concourse.bass as bass
    import concourse.tile as tile
    from concourse import mybir
    from concourse.vector_clock import ScopedClock
    from concourse.tile_rust import add_dep_helper

    # --- workaround: this walrus build rejects >1 sync-wait on CTRL-class
    # instructions; split the Tile tail-drain waits into 1-wait NOPs. ---
    def patched_drain_and_barrier(self, tick_clock, wait_clock):
        maxw = 1
        carrier = self.nc.sync.nop()
        wait_clock.add_sem_waits(
            carrier.ins, ScopedClock({None: tick_clock.global_clock})
        )
        si = carrier.ins.sync_info
        waits = list(si.on_wait) if si is not None else []
        if len(waits) > maxw:
            si.on_wait = waits[:maxw]
            carrier.ins.sync_info = si
            rest = waits[maxw:]
            for i in range(0, len(rest), maxw):
                n = self.nc.sync.nop()
                n.ins.sync_info = mybir.SyncInfo(
                    on_wait=rest[i : i + maxw], on_update=[]
                )
        self.nc.sync.drain()
        self.nc.all_engine_barrier()
        assert self.sems is not None
        popped = self.nc._tile_sem_poison_stack.pop()
        assert popped is self._sem_poison
        self.nc.clear_and_free_semaphores(list(self.sems.allocated().values()))
        self.nc.all_engine_barrier()

    tile.TileContext._drain_and_barrier = patched_drain_and_barrier

    # --- workaround #2: the same walrus build accepts at most ONE sync wait
    # on ANY instruction.  Post-process the serialized BIR: hoist excess
    # waits onto single-wait NoOps inserted just before, on the same engine
    # (same program point, so semantics are unchanged). ---
    import orjson

    def _split_waits_json(raw: bytes) -> bytes:
        j = orjson.loads(raw)
        n = 0
        changed = False
        for fn in j["functions"]:
            for bb in fn["blocks"]:
                out = []
                for ins in bb["instructions"]:
                    si = ins.get("sync_info")
                    waits = si.get("on_wait") if si else None
                    if waits and len(waits) > 1:
                        changed = True
                        for wt in waits[:-1]:
                            n += 1
                            out.append(
                                {
                                    "debug": ins.get("debug", 0),
                                    "engine": ins["engine"],
                                    "ins": [],
                                    "outs": [],
                                    "name": f"WSPL-{n}",
                                    "opcode": "NoOp",
                                    "sync_info": {"on_update": [], "on_wait": [wt]},
                                }
                            )
                        si["on_wait"] = [waits[-1]]
                    out.append(ins)
                bb["instructions"] = out
        return orjson.dumps(j) if changed else raw

    if not getattr(bass.Bass.to_json_bytes, "_wait_split", False):
        _orig_tjb = bass.Bass.to_json_bytes

        def patched_to_json_bytes(self):
            return _split_waits_json(_orig_tjb(self))

        patched_to_json_bytes._wait_split = True
        bass.Bass.to_json_bytes = patched_to_json_bytes

    f32 = mybir.dt.float32
    bf16 = mybir.dt.bfloat16
    i16 = mybir.dt.int16
    AF = mybir.ActivationFunctionType

    nc = bass.Bass()

    # ---- parameters ----
    # Packed inputs (host-prepared layouts; see _make_in_maps):
    #  xh    [B, 2, 128, XROWS*WP]   bf16  image, channels on partitions
    #  xtin  [B, XROWS, WP, 256]     bf16  image pre-transposed (w on partitions)
    #  bfpk  [128, BFPK]             bf16  ident|mask|shmat|encT|compT
    #  f32pk [CENC, F32PK]           f32   sel|selT|bn params
    #  lsidx [LCH, CENC]             i16   local_scatter indices
    BFPK = 128 + 660 + K_UP * LCH + 9 * CENC + 2 * CMID
    F32PK = 4 + CENC + 3
    xtin = nc.declare_dram_parameter(
        "xtin", [B, XROWS, WP, 256], bf16, isOutput=False
    )
    xc1 = nc.declare_dram_parameter(
        "xc1", [B, 128, 2, 2, 320], bf16, isOutput=False
    )
    bfpk = nc.declare_dram_parameter("bfpk", [128, BFPK], bf16, isOutput=False)
    f32pk = nc.declare_dram_parameter("f32pk", [CENC, F32PK], f32, isOutput=False)
    lsidx = nc.declare_dram_parameter("lsidx", [LCH, CENC], i16, isOutput=False)

    out = nc.declare_dram_parameter(
        "out", [B, C, 2 * HS, 2 * W], f32, isOutput=True
    )

    def dram_ap(param, offset, dims):
        return bass.AP(tensor=param, offset=offset, ap=[list(d) for d in dims])

    with tile.TileContext(nc) as tc:
        import contextlib

        ctx = contextlib.ExitStack()
        const = ctx.enter_context(tc.tile_pool(name="const", bufs=1))
        sm = ctx.enter_context(tc.tile_pool(name="sm", bufs=2))
        dp = ctx.enter_context(tc.tile_pool(name="dp", bufs=4))
        btp = ctx.enter_context(tc.tile_pool(name="btp", bufs=16))
        op = ctx.enter_context(tc.tile_pool(name="op", bufs=6))
        ps_big = ctx.enter_context(tc.tile_pool(name="ps_big", bufs=2, space="PSUM"))
        ps_bf = ctx.enter_context(tc.tile_pool(name="ps_bf", bufs=1, space="PSUM"))
        ps_sh = ctx.enter_context(tc.tile_pool(name="ps_sh", bufs=2, space="PSUM"))
        ps_e = ctx.enter_context(tc.tile_pool(name="ps_e", bufs=3, space="PSUM"))

        # ---- packed constants in SBUF (sync queue; scalar queue kept free
        # for ACT compute + out-DMAs).  conv1(b0) critical path first: weights
        # (tail of bfpk) as a separate early DMA, then f32pk, then the rest.
        bf_sb = const.tile([128, BFPK], bf16, tag="bfpk")
        W_OFF = 128 + 660 + K_UP * LCH  # encT|compT live at the tail
        nc.sync.dma_start(
            out=bf_sb[:, W_OFF:BFPK],
            in_=dram_ap(
                bfpk, W_OFF, [[BFPK, 128], [1, BFPK - W_OFF]]
            ),
        )
        f32_sb = const.tile([CENC, F32PK], f32, tag="f32pk")
        nc.sync.dma_start(out=f32_sb[:, :], in_=f32pk[:, :])

        o_id = 0
        ident_sb = bf_sb[:, 0:128]
        o_id += 128
        mask_v = bf_sb[0:CMID, o_id : o_id + 660].rearrange(
            "p (a b) -> p a b", b=66
        )
        o_id += 660
        sh_all = bf_sb[0:W, o_id : o_id + K_UP * LCH].rearrange(
            "p (a b) -> p a b", b=LCH
        )
        o_id += K_UP * LCH
        enc_bf = []
        for j in range(9):
            enc_bf.append(bf_sb[0:CMID, o_id : o_id + CENC])
            o_id += CENC
        comp_bf = []
        for ct in range(2):
            comp_bf.append(bf_sb[:, o_id : o_id + CMID])
            o_id += CMID

        sel_sb = f32_sb[:, 0:4]
        selT_sb = f32_sb[0:4, 4 : 4 + CENC]
        # bn folded into weights host-side; shifts + clipped power as columns
        shift1_ap = f32_sb[0:CMID, 4 + CENC : 4 + CENC + 1]
        shift2_ap = f32_sb[:, 4 + CENC + 1 : 4 + CENC + 2]
        pb_sb = f32_sb[:, 4 + CENC + 2 : 4 + CENC + 3]

        # ---- Y1 tiles (zeroed once; borders stay zero) ----
        y1 = []
        for b in range(B):
            t = const.tile([CMID, 10, 66], bf16, tag=f"y1_{b}")
            nc.vector.memset(t[:, :, :], 0.0)
            y1.append(t)

        # ---- per-batch X loads (sync queue, emitted lazily per b) ----
        xts_all = [None, None]
        xc1_sb = [None, None]

        def xload(b, rest=False):
            if not rest:
                t = const.tile([128, 2, 2, 320], bf16, tag=f"xc1{b}")
                nc.sync.dma_start(
                    out=t[:, :, :, :],
                    in_=dram_ap(xc1, b * 128 * 1280, [[1280, 128], [1, 1280]]),
                )
                xc1_sb[b] = t
                return
            t = const.tile([WP, XROWS, 256], bf16, tag=f"xts{b}")
            nc.sync.dma_start(
                out=t[:, :, :],
                in_=dram_ap(
                    xtin,
                    b * XROWS * WP * 256,
                    [[256, WP], [WP * 256, XROWS], [1, 256]],
                ),
            )
            xts_all[b] = t

        en_sbs = {}
        bts_alls = {}

        def prep(b):
            # ===== conv1x1 + bn1 + relu (contiguous host-packed input) =====
            for half in range(2):
                pcb = ps_big.tile([CENC, HS * W], f32, tag="big")
                pc = pcb[0:CMID, 0:320]
                for ct in range(2):
                    nc.tensor.matmul(
                        pc,
                        comp_bf[ct],
                        xc1_sb[b][:, half, ct, :],
                        start=(ct == 0),
                        stop=(ct == 1),
                    )
                nc.vector.tensor_scalar(
                    y1[b][:, 5 * half : 5 * half + 5, 1 : 1 + W],
                    pc,
                    shift1_ap,
                    0.0,
                    mybir.AluOpType.add,
                    mybir.AluOpType.max,
                )
            # zero out-of-image rows / padding cols
            nc.vector.tensor_mul(y1[b][:, :, :], y1[b][:, :, :], mask_v)

            # ===== conv3x3 + bn2 =====
            pc3 = ps_big.tile([CENC, HS * W], f32, tag="big")
            jj = 0
            for dy in (-1, 0, 1):
                for dx in (-1, 0, 1):
                    nc.tensor.matmul(
                        pc3[:, :],
                        enc_bf[jj],
                        y1[b][:, 1 + dy : 9 + dy, 1 + dx : 1 + dx + W],
                        start=(jj == 0),
                        stop=(jj == 8),
                    )
                    jj += 1
            # ===== bn2-shift + clip (fused DVE) + power + softmax numerator ==
            w_sb = sm.tile([CENC, HS * W], f32, tag="w")
            nc.vector.tensor_scalar(
                w_sb[:, :],
                pc3[:, :],
                shift2_ap,
                1e-5,
                mybir.AluOpType.add,
                mybir.AluOpType.max,
            )
            nc.scalar.activation(w_sb[:, :], w_sb[:, :], AF.Ln)
            nc.scalar.activation(w_sb[:, :], w_sb[:, :], AF.Exp, scale=pb_sb)
            e_sb = sm.tile([CENC, HS * W], f32, tag="e")
            nc.scalar.activation(e_sb[:, :], w_sb[:, :], AF.Exp)

            # ===== tap-sums, reciprocal, broadcast, normalize =====
            psb = ps_big.tile([CENC, HS * W], f32, tag="big")
            ps = psb[0:4, :]
            nc.tensor.matmul(ps, sel_sb, e_sb[:, :], start=True, stop=True)
            r4_sb = sm.tile([4, HS * W], f32, tag="r4")
            nc.vector.reciprocal_approx_fast(r4_sb[:, :], ps)
            rb_ps = ps_big.tile([CENC, HS * W], f32, tag="big")
            nc.tensor.matmul(
                rb_ps[:, :], selT_sb, r4_sb[:, :], start=True, stop=True
            )
            en_sb = const.tile([CENC, HS, W], bf16, tag=f"en{b}")
            nc.vector.tensor_mul(
                en_sb[:, :, :],
                e_sb[:, :].rearrange("p (a b) -> p a b", b=W),
                rb_ps[:, :].rearrange("p (a b) -> p a b", b=W),
            )
            en_sbs[b] = en_sb

        def band(b):
            # ===== banded-matrix build (h-quads): 4 transposes into one psum,
            # 5 quad-fused shift matmuls, then per-h data copy + local_scatter
            # into per-pair [LCH, 2*BTN] tiles =====
            en_sb = en_sbs[b]
            bts_all = []
            for hq in range(HS // 4):
                tpt = ps_bf.tile([W, 4 * CENC], bf16, tag="bf")
                for hh in range(4):
                    nc.tensor.transpose(
                        tpt[:, CENC * hh : CENC * (hh + 1)],
                        en_sb[:, 4 * hq + hh, :],
                        ident_sb[0:CENC, 0:CENC],
                    )
                tp_q = dp.tile([W, 4 * CENC], bf16, tag="tps")
                nc.vector.tensor_copy(tp_q[:, :], tpt[:, :])
                # cols of tp_q: hh*100 + ki*20 + kj*4 + u
                tp_v = tp_q[:, :].rearrange(
                    "p (hh ki r) -> p hh ki r", hh=4, r=20
                )
                sh_ps = ps_sh.tile([LCH, 4 * CENC], f32, tag="sh")
                sh_v = sh_ps[:, :].rearrange("p (hh r) -> p hh r", hh=4)
                for s in range(K_UP):
                    # out cols (hh, 20) at base s*20; rhs (hh, ki, u) base s*4
                    nc.tensor.matmul(
                        sh_v[:, :, 20 * s : 20 * (s + 1)],
                        sh_all[:, s, :],
                        tp_v[:, :, :, 4 * s : 4 * (s + 1)],
                        start=True,
                        stop=True,
                    )
                for hp2 in range(2):
                    btsp = btp.tile([LCH, 2 * BTN], bf16, tag="bts")
                    for hh2 in range(2):
                        hh = 2 * hp2 + hh2
                        data_sb = dp.tile([LCH, CENC], bf16, tag="data")
                        nc.vector.tensor_copy(
                            data_sb[:, :], sh_ps[:, CENC * hh : CENC * (hh + 1)]
                        )
                        nc.gpsimd.local_scatter(
                            out_ap=btsp[:, BTN * hh2 : BTN * (hh2 + 1)],
                            data_ap=data_sb[:, :],
                            idxs_ap=lsidx_sb[:, :],
                            channels=LCH,
                            num_elems=BTN,
                            num_idxs=CENC,
                        )
                    bts_all.append(btsp)
            bts_alls[b] = bts_all

        def eins(b):
            # ===== banded einsum: h-pair-fused matmuls (6 per pair per ct),
            # one out DMA per (pair, ct) =====
            bts_all = bts_alls[b]
            for hp in range(HS // 2):
                ha = 2 * hp
                btsp = bts_all[hp]
                for ct in range(2):
                    pe = ps_e.tile([128, 512], f32, tag="pe")
                    cs = ct * 128
                    # fused strips jj=1..4: regions (ha@ki=jj, hb@ki=jj-1)
                    base_ap = btsp[0:WP, 0:256]
                    for jj in range(1, K_UP):
                        rhs = bass.AP(
                            tensor=base_ap.tensor,
                            offset=base_ap.offset + jj * 256,
                            ap=[list(base_ap.ap[0]), [BTN - 256, 2], [1, 256]],
                        )
                        nc.tensor.matmul(
                            pe[:, :],
                            xts_all[b][:, ha + jj, cs : cs + 128],
                            rhs,
                            start=(jj == 1),
                            stop=False,
                        )
                    # single strip jj=0 -> region A only
                    nc.tensor.matmul(
                        pe[:, 0:256],
                        xts_all[b][:, ha, cs : cs + 128],
                        btsp[0:WP, 0:256],
                        start=False,
                        stop=True,
                    )
                    # single strip jj=5 -> region B only
                    nc.tensor.matmul(
                        pe[:, 256:512],
                        xts_all[b][:, ha + 5, cs : cs + 128],
                        btsp[0:WP, BTN + 4 * 256 : BTN + 5 * 256],
                        start=False,
                        stop=True,
                    )
                    o_sb = op.tile([128, 512], f32, tag="osb")
                    if ct == 0:
                        nc.scalar.activation(o_sb[:, :], pe[:, :], AF.Identity)
                    else:
                        nc.vector.tensor_copy(o_sb[:, :], pe[:, :])
                    oeng = nc.sync if ct == 0 else nc.scalar
                    oeng.dma_start(
                        out=dram_ap(
                            out,
                            b * C * 2 * HS * 2 * W
                            + ct * 128 * 2 * HS * 2 * W
                            + 4 * hp * 2 * W,
                            [[2 * HS * 2 * W, 128], [1, 512]],
                        ),
                        in_=o_sb[:, :],
                    )

        xload(0)
        # remaining constants + idx after the conv1(b0)-critical loads
        nc.sync.dma_start(
            out=bf_sb[:, 0:W_OFF],
            in_=dram_ap(bfpk, 0, [[BFPK, 128], [1, W_OFF]]),
        )
        lsidx_sb = const.tile([LCH, CENC], i16, tag="lsidx")
        nc.sync.dma_start(out=lsidx_sb[:, :], in_=lsidx[:, :])
        xload(1)
        xload(0, rest=True)
        xload(1, rest=True)
        prep(0)
        prep(1)
        band(0)
        band(1)
        eins(0)
        eins(1)

        ctx.close()

    # ---- Bacc-style finishing passes: library loads + ISA assembly ----
    from concourse.library_config import all_libraries, standard
    import bass_rust as _bass_rust

    lib_mask = {}
    for lib in all_libraries:
        for it in lib.instructions:
            lib_mask[it] = lib_mask.get(it, 0) | (1 << lib.index)
    _bass_rust.insert_library_loads(nc, lib_mask, len(all_libraries), standard.index)
    mybir.codegen_inst_isa_subclasses(nc)

    return nc


def _get_nc():
    if "nc" not in _STATE:
        _STATE["nc"] = _build_nc()
    return _STATE["nc"]


def _make_in_maps(inputs):
    bf16 = ml_dtypes.bfloat16
    BFPK = 128 + 660 + K_UP * LCH + 9 * CENC + 2 * CMID
    F32PK = 4 + CENC + 3
    X = np.asarray(inputs["X"], dtype=np.float32)
    Xp = np.pad(X, ((0, 0), (0, 0), (2, 2), (2, 2)))

    sel = np.zeros((CENC, 4), np.float32)
    for p in range(CENC):
        sel[p, p % 4] = 1.0
    shmat = np.zeros((K_UP, W, LCH), np.float32)
    for s in range(K_UP):
        for w in range(W):
            shmat[s, w, w + s] = 1.0
    lsidx = np.full((LCH, CENC), -1, np.int16)
    for p in range(WP):
        for s in range(K_UP):
            w = p - s
            if 0 <= w < W:
                for ki in range(K_UP):
                    for u in range(4):
                        ry, rx = u // 2, u % 2
                        c = s * 20 + ki * 4 + u
                        lsidx[p, c] = ki * 256 + ry * 128 + 2 * w + rx
    # fold eval-mode batchnorms into the conv weights
    inv1 = np.asarray(inputs["comp_gamma"], np.float32) / np.sqrt(
        np.asarray(inputs["comp_var"], np.float32) + 1e-5
    )
    shift1 = (
        np.asarray(inputs["comp_beta"], np.float32)
        - np.asarray(inputs["comp_mean"], np.float32) * inv1
    )
    inv2 = np.asarray(inputs["enc_gamma"], np.float32) / np.sqrt(
        np.asarray(inputs["enc_var"], np.float32) + 1e-5
    )
    shift2 = (
        np.asarray(inputs["enc_beta"], np.float32)
        - np.asarray(inputs["enc_mean"], np.float32) * inv2
    )
    comp_wT = (
        np.asarray(inputs["comp_w"], np.float32)[:, :, 0, 0] * inv1[:, None]
    ).T.reshape(2, 128, CMID)
    enc_wT = (
        np.asarray(inputs["enc_w"], np.float32) * inv2[:, None, None, None]
    ).reshape(CENC, CMID, 9).transpose(2, 1, 0)

    # bf16 pack: ident | y1mask(per-core) | shmat | encT | compT
    bfpk = np.zeros((128, BFPK), np.float32)
    o = 0
    bfpk[:, o : o + 128] = np.eye(128)
    o_mask = o = o + 128
    o += 660
    bfpk[0:W, o : o + K_UP * LCH] = shmat.transpose(1, 0, 2).reshape(W, K_UP * LCH)
    o += K_UP * LCH
    bfpk[0:CMID, o : o + 9 * CENC] = enc_wT.transpose(1, 0, 2).reshape(
        CMID, 9 * CENC
    )
    o += 9 * CENC
    bfpk[:, o : o + 2 * CMID] = comp_wT.transpose(1, 0, 2).reshape(128, 2 * CMID)

    # f32 pack: sel | selT | shift1 | shift2 | clipped power (broadcast)
    f32pk = np.zeros((CENC, F32PK), np.float32)
    f32pk[:, 0:4] = sel
    f32pk[0:4, 4 : 4 + CENC] = sel.T
    f32pk[0:CMID, 4 + CENC] = shift1
    f32pk[:, 4 + CENC + 1] = shift2
    f32pk[:, 4 + CENC + 2] = max(
        float(np.asarray(inputs["power_p"], np.float32)[0]), 1e-5
    )

    common = {
        "f32pk": f32pk,
        "lsidx": lsidx,
    }
    in_maps = []
    for core in range(N_CORES):
        r0 = HS * core
        xh4 = np.ascontiguousarray(Xp[:, :, r0 : r0 + XROWS, :]).astype(bf16)
        mask = np.zeros((10, 66), np.float32)
        for rr in range(10):
            grow = r0 - 1 + rr
            if 0 <= grow < H:
                mask[rr, 1 : 1 + W] = 1.0
        bfpk_c = bfpk.copy()
        bfpk_c[0:CMID, o_mask : o_mask + 660] = mask.reshape(1, 660)
        xc1 = (
            xh4[:, :, 1:11, 2:66]
            .reshape(B, 2, 128, 2, 5, 64)
            .transpose(0, 2, 3, 1, 4, 5)
            .reshape(B, 128, 2, 2, 320)
        )
        m = dict(common)
        m["xc1"] = np.ascontiguousarray(xc1)
        m["xtin"] = np.ascontiguousarray(xh4.transpose(0, 2, 3, 1))
        m["bfpk"] = bfpk_c.astype(bf16)
        in_maps.append(m)
    return in_maps


def _run(inputs, trace=False):
    from concourse.bass_utils import run_bass_kernel_spmd

    if trace:
        import sys, os
        sys.path.insert(0, os.path.dirname(os.path.abspath(__file__)))
        import hookshim  # noqa: F401

    nc = _get_nc()
    in_maps = _make_in_maps(inputs)
    res = run_bass_kernel_spmd(
        nc, in_maps, core_ids=list(range(N_CORES)), trace=trace
    )
    out = np.concatenate([res.results[c]["out"] for c in range(N_CORES)], axis=2)
    return out, res


def kernel(**inputs):
    out, _ = _run(inputs, trace=False)
    return out
